# revision 86
# baseline (speedup 1.0000x reference)
"""Causal multi-head attention (nn.MultiHeadAttention, B=2, S=2048, D=1024, H=16)
on 8 Trainium2 NeuronCores.

Sharding: core c = (batch b = c // 4, head-group hg = c % 4); data parallel on
batch, tensor parallel over 4-head groups (qkv weight columns / proj weight
rows). Each core computes its partial output projection [2048, 1024] in bf16;
the host sums the 4 head-group partials per batch and adds proj_b plus the
v-bias correction (softmax weights sum to 1, so the V bias contributes exactly
bv @ proj_w to the output — applied host-side).

Per-core device kernel (Bass/Tile):
  - x arrives pre-transposed from the host as fp8(e4m3) hi/lo residual planes
    x8[128, 2, 8, S] (plane 0 = e4m3(x^T), plane 1 = e4m3(x^T - plane0));
    qkv weights likewise, pre-scaled by 32 so the residual stays above the
    e4m3 subnormal cutoff. QKV projections run as fp8 DoubleRow matmuls
    (2 contraction chunks per instruction at 0.5 cycles/row): the 3-term
    expansion hi*hi + lo*hi + hi*lo reconstructs the product to ~1.3e-3,
    at 0.75x the bf16 PE cost. The 1/32 descale folds into the PSUM->SBUF
    bias-add (tensor_scalar mult+add).
  - Q^T/K^T [hd, s] bf16 with two heads stacked per 128 partitions; V in
    natural [s, hd] bf16 layout with an appended ones-column
  - scores computed transposed S^T[k, q] = K @ Q^T in bf16 (exact layout the
    flipped PV consumes as its stationary operand)
  - exp on ScalarE (no max subtraction: scores ~ N(0,1) by construction),
    output directly in bf16
  - causal masking: bf16 0/1 multiplies (VectorE) for the two lower diagonal
    blocks; gpsimd affine_select for the two upper (reduced-width) blocks
  - PV flipped: stationary = probability tile [k=128, q=128], moving =
    V|ones [k=128, 66] -> output O[q, hd] natural per q-tile at 66 rows per
    k-tile (half the PE cost of the O^T orientation), with the softmax
    denominator landing in column 64 per-partition -> cheap [128,1]
    reciprocal + per-partition tensor_scalar normalize (no partition
    broadcast)
  - normalized O transposed back to O^T[hd, q] on the PE (bf16, 128 rows per
    q-tile) for the output projection with bf16 proj weights
  - projection staged to SBUF on VectorE/ScalarE (gpsimd cannot read PSUM on
    hardware), partials DMA'd out in bf16
  - schedule: heads software-pipelined with PV lagging scores by 3 heads, a
    priority/filler queue system paces next-window QKV and floating
    projection work into ScalarE's exp latency, plus a PE warmup chain
    under the initial DMAs
"""

import sys
from contextlib import ExitStack

import numpy as np

for _p in ("/opt/trn_rl_repo", "/root/.axon_site/_ro/trn_rl_repo"):
    if _p not in sys.path:
        sys.path.append(_p)

B = 2
S = 2048
D = 1024
H_TOT = 16
HPC = 4             # heads per core
HD = 64
NCHUNK = D // 128   # 8 contraction chunks
NQW = S // 512      # 4 q-windows
NKT = S // 128      # 16 k-tiles
N_CORES = 8
AW = 32.0           # host-side weight prescale (descaled on device)


# --------------------------------------------------------------------------
# device kernel builder
# --------------------------------------------------------------------------

def _build_body(ctx, tc, x8, w8q, w8k, w8v, wp, ident, dmask, bq, bk, out_part):
    import concourse.tile as tile  # noqa: F401
    from concourse import mybir

    F32 = mybir.dt.float32
    BF16 = mybir.dt.bfloat16
    F8 = mybir.dt.float8e4
    DR = mybir.MatmulPerfMode.DoubleRow
    MUL = mybir.AluOpType.mult
    ADD = mybir.AluOpType.add
    nc = tc.nc

    consts = ctx.enter_context(tc.tile_pool(name="consts", bufs=1))
    persist = ctx.enter_context(tc.tile_pool(name="persist", bufs=1))
    pt_pool = ctx.enter_context(tc.tile_pool(name="pt", bufs=48))
    pt2_pool = ctx.enter_context(tc.tile_pool(name="pt2", bufs=7))
    small = ctx.enter_context(tc.tile_pool(name="small", bufs=4))
    stage = ctx.enter_context(tc.tile_pool(name="stage", bufs=3))
    pS = ctx.enter_context(tc.tile_pool(name="pS", bufs=2, space="PSUM"))
    pP = ctx.enter_context(tc.tile_pool(name="pP", bufs=2, space="PSUM"))
    pM = ctx.enter_context(tc.tile_pool(name="pM", bufs=2, space="PSUM"))

    # ---- persistent activations ----
    x8_sb = persist.tile([128, 2, NCHUNK, S], F8)
    qt = [persist.tile([128, S], BF16, tag=f"qt{i}", name=f"qt{i}") for i in range(2)]
    kt_ = [persist.tile([128, S], BF16, tag=f"kt{i}", name=f"kt{i}") for i in range(2)]
    ot = [persist.tile([128, S], BF16, tag=f"ot{i}", name=f"ot{i}") for i in range(2)]
    v_sb = persist.tile([128, HPC, NKT, 66], BF16)
    nc.gpsimd.memset(v_sb[:, :, :, 64:65], 1.0)
    nc.gpsimd.memset(v_sb[:, :, :, 65:66], 0.0)

    # ---- constants ----
    ident_sb = consts.tile([128, 128], BF16)
    w8q_sb = consts.tile([128, 2, NCHUNK, 256], F8)
    w8k_sb = consts.tile([128, 2, NCHUNK, 256], F8)
    w8v_sb = consts.tile([128, 2, NCHUNK, 256], F8)
    wp_sb = consts.tile([128, 2, D], BF16)
    dmask_sb = consts.tile([128, 4, 512], BF16)
    bq_sb = consts.tile([128, 2], F32)
    bk_sb = consts.tile([128, 2], F32)

    # w8q first (gates the first QK matmul), then x in 512-s chunks so the
    # first unit can start ~4.5us in; the rest streams behind compute.
    nc.sync.dma_start(w8q_sb[:], w8q[:])
    nc.sync.dma_start(x8_sb[:, :, :, 0:512], x8[:, :, :, 0:512])
    nc.sync.dma_start(w8k_sb[:], w8k[:])
    nc.sync.dma_start(bq_sb[:], bq[:])
    nc.sync.dma_start(bk_sb[:], bk[:])
    nc.sync.dma_start(ident_sb[:], ident[:])
    nc.sync.dma_start(w8v_sb[:], w8v[:])
    nc.sync.dma_start(x8_sb[:, :, :, 512:1024], x8[:, :, :, 512:1024])
    nc.sync.dma_start(dmask_sb[:], dmask.rearrange("p (j q) -> p j q", j=4))
    nc.sync.dma_start(x8_sb[:, :, :, 1024:1536], x8[:, :, :, 1024:1536])
    nc.sync.dma_start(wp_sb[:], wp[:])
    nc.sync.dma_start(x8_sb[:, :, :, 1536:2048], x8[:, :, :, 1536:2048])

    # fp8 3-term residual product: hi*hi + lo*hi + hi*lo, each DoubleRow
    # instruction covering a (hi/lo plane, chunk pair).
    TERMS = ((0, 0), (1, 0), (0, 1))

    def warmup():
        """Dummy matmul chain that keeps the PE continuously busy (and its
        p-state ramping) while the first input DMAs land."""
        junk = small.tile([128, 512], BF16, tag="junk", bufs=1)
        nc.gpsimd.memset(junk[:], 0.5)
        jps = pM.tile([128, 512], F32, tag="pM", name="jps")
        for _ in range(12):
            nc.tensor.matmul(jps[:], junk[:, 0:128], junk[:],
                             start=True, stop=True)

    def qkv_qk_unit(w, wsel, gh):
        """Q^T/K^T rows for head-pair gh, s-window w (12 DoubleRow matmuls)."""
        w_sb, dsts, b_sb = ((w8q_sb, qt, bq_sb), (w8k_sb, kt_, bk_sb))[wsel]
        s0 = w * 512
        ps_q = pM.tile([128, 512], F32, tag="pM", name="ps_q")
        n = 0
        for tw, tx in TERMS:
            for cp in range(4):
                nc.tensor.matmul(
                    ps_q[:],
                    w_sb[:, tw, 2 * cp:2 * cp + 2, gh * 128:(gh + 1) * 128],
                    x8_sb[:, tx, 2 * cp:2 * cp + 2, s0:s0 + 512],
                    start=(n == 0),
                    stop=(n == 11),
                    perf_mode=DR,
                )
                n += 1
        nc.vector.tensor_scalar(
            out=dsts[gh][:, s0:s0 + 512], in0=ps_q[:],
            scalar1=1.0 / AW, scalar2=b_sb[:, gh:gh + 1],
            op0=MUL, op1=ADD,
        )

    def qkv_v_unit(st):
        """V rows for one s-tile."""
        ps_v = pM.tile([128, 512], F32, tag="pM", name="ps_v")
        n = 0
        for tw, tx in TERMS:
            for cp in range(4):
                nc.tensor.matmul(
                    ps_v[:, 0:256],
                    x8_sb[:, tw, 2 * cp:2 * cp + 2, st * 128:(st + 1) * 128],
                    w8v_sb[:, tx, 2 * cp:2 * cp + 2, :],
                    start=(n == 0),
                    stop=(n == 11),
                    perf_mode=DR,
                )
                n += 1
        nc.vector.tensor_scalar(
            out=v_sb[:, :, st, 0:64],
            in0=ps_v[:, 0:256].rearrange("p (h e) -> p h e", h=HPC),
            scalar1=1.0 / AW, scalar2=None, op0=MUL,
        )

    def scores_head(qw, h, tap):
        """S^T = K Q^T pair by pair, exp to bf16, causal-mask. Returns the
        per-k-tile (tile, col, qmin) list the flipped PV slices from.
        `tap()` is called after each pair so the scheduler can wedge
        PE filler work into the exp pipeline."""
        ha, hp = h // 2, (h % 2) * 64
        qs = qt[ha][hp:hp + 64, qw * 512:(qw + 1) * 512]
        pts = [None] * (4 * qw + 4)
        # Diagonal work first: PV consumes every k-tile of the window, so the
        # blocks exp'd last become PV's stall points — put the diagonal (and
        # reduced) pairs at the head of the exp queue and let PV accumulate
        # in the same order.
        # reduced-width diagonal pair: k-tile 4qw+2 covers q in [256, 512),
        # k-tile 4qw+3 only q in [384, 512)
        ps_s = pS.tile([128, 1024], F32, tag="pS", name="ps_s")
        nc.tensor.matmul(
            ps_s[:, 0:256],
            kt_[ha][hp:hp + 64, (4 * qw + 2) * 128:(4 * qw + 3) * 128],
            qs[:, 256:512],
            start=True,
            stop=True,
        )
        nc.tensor.matmul(
            ps_s[:, 256:384],
            kt_[ha][hp:hp + 64, (4 * qw + 3) * 128:(4 * qw + 4) * 128],
            qs[:, 384:512],
            start=True,
            stop=True,
        )
        pt2 = pt2_pool.tile([128, 384], BF16, tag="pt2")
        nc.scalar.activation(pt2[:], ps_s[:, 0:384],
                             mybir.ActivationFunctionType.Exp, scale=0.125)
        # keep where (q - 256) >= k  /  (q - 384) >= k
        nc.gpsimd.affine_select(
            out=pt2[:, 0:256], in_=pt2[:, 0:256],
            compare_op=mybir.AluOpType.is_ge, fill=0.0,
            base=0, channel_multiplier=-1, pattern=[[1, 256]],
        )
        nc.gpsimd.affine_select(
            out=pt2[:, 256:384], in_=pt2[:, 256:384],
            compare_op=mybir.AluOpType.is_ge, fill=0.0,
            base=0, channel_multiplier=-1, pattern=[[1, 128]],
        )
        pts[4 * qw + 2] = (pt2, 0, 256)
        pts[4 * qw + 3] = (pt2, 256, 384)
        tap()
        for pair in [2 * qw] + list(range(2 * qw)):  # diagonal pair first
            kt0 = 2 * pair
            diag = pair == 2 * qw
            ps_s = pS.tile([128, 1024], F32, tag="pS", name="ps_s")
            nc.tensor.matmul(
                ps_s[:, 0:512],
                kt_[ha][hp:hp + 64, kt0 * 128:(kt0 + 1) * 128],
                qs,
                start=True,
                stop=True,
            )
            # the upper diagonal block only attends q in [128, 512)
            w1 = 384 if diag else 512
            nc.tensor.matmul(
                ps_s[:, 512:512 + w1],
                kt_[ha][hp:hp + 64, (kt0 + 1) * 128:(kt0 + 2) * 128],
                qs[:, 512 - w1:512],
                start=True,
                stop=True,
            )
            pt = pt_pool.tile([128, 1024], BF16, tag="pt")
            nc.scalar.activation(pt[:, 0:512 + w1], ps_s[:, 0:512 + w1],
                                 mybir.ActivationFunctionType.Exp, scale=0.125)
            if diag:  # zero the strictly-upper triangles (deferred so the
                # older pv's reciprocal/normalize go first in the DVE queue)
                def masks(pt=pt):
                    nc.vector.tensor_mul(pt[:, 0:512], pt[:, 0:512],
                                         dmask_sb[:, 0, :])
                    nc.vector.tensor_mul(pt[:, 512:896], pt[:, 512:896],
                                         dmask_sb[:, 1, 128:512])
                mask_tasks.append(masks)
            pts[kt0] = (pt, 0, 0)
            pts[kt0 + 1] = (pt, 512, 512 - w1)
            tap()
        return pts

    def pv_half(qw, h, pts, tp):
        """Flipped PV for one q-tile pair: O[q, hd] natural + denominator
        col, both accumulation groups in one PSUM bank, then reciprocal,
        per-q-tile normalize, and PE transposes back to O^T rows."""
        ha, hp = h // 2, (h % 2) * 64
        po = pP.tile([128, 512], F32, tag="pP", name="po")
        po = po.rearrange("p (i e) -> p i e", i=2)
        for i in range(2):
            tt = tp * 2 + i
            nk = 4 * qw + tt + 1
            # accumulate in the order scores_head exp'd the tiles
            order = [k for k in (4 * qw + 2, 4 * qw + 3, 4 * qw, 4 * qw + 1)
                     if k < nk] + list(range(2 * qw * 2))
            for n, kti in enumerate(order):
                pt, col, qmin = pts[kti]
                lhs = pt[:, col + tt * 128 - qmin:col + (tt + 1) * 128 - qmin]
                nc.tensor.matmul(
                    po[:, i, 0:65],
                    lhs,
                    v_sb[:, h, kti, 0:65],
                    start=(n == 0),
                    stop=(n == nk - 1),
                    skip_group_check=True,
                )
        rec = small.tile([128, 2, 1], F32, tag="rec")
        nc.vector.reciprocal(rec[:], po[:, :, 64:65])
        o_sb = small.tile([128, 2, 64], BF16, tag="osb", bufs=8)
        for i in range(2):
            nc.vector.tensor_scalar(
                out=o_sb[:, i, :], in0=po[:, i, 0:64],
                scalar1=rec[:, i, :], scalar2=None, op0=MUL,
            )

        def emit_oT(ha=ha, hp=hp, qw=qw, tp=tp, o_sb=o_sb):
            oT = pM.tile([128, 2, 128], BF16, tag="pM", name="oT")
            for i in range(2):
                nc.tensor.transpose(oT[0:64, i, :], o_sb[:, i, :], ident_sb[:])
            nc.vector.tensor_copy(
                ot[ha][hp:hp + 64,
                       qw * 512 + tp * 256:qw * 512 + tp * 256 + 256],
                oT[0:64, :, :],
            )
        # defer one pop so the transpose never waits on the normalize
        ot_tasks.append(emit_oT)
        if len(ot_tasks) > 1:
            ot_tasks.popleft()()

    def proj_half(st, nh, copy_eng="dve"):
        """Half an output-projection s-tile (one PSUM bank)."""
        ps_p = pM.tile([128, 512], F32, tag="pM", name="ps_p")
        for ci in range(2):
            nc.tensor.matmul(
                ps_p[:],
                ot[ci][:, st * 128:(st + 1) * 128],
                wp_sb[:, ci, nh * 512:(nh + 1) * 512],
                start=(ci == 0),
                stop=(ci == 1),
            )
        if nh == 0:
            stg = stage.tile([128, D], BF16, tag="stg")
            stgs[st] = stg
        else:
            stg = stgs.pop(st)
        half = stg[:, nh * 512:(nh + 1) * 512]
        if copy_eng == "dve":
            nc.vector.tensor_copy(half, ps_p[:])
        else:
            nc.scalar.activation(half, ps_p[:],
                                 mybir.ActivationFunctionType.Copy)
        if nh == 1:
            nc.sync.dma_start(out_part[st * 128:(st + 1) * 128, :], stg[:])

    # ---- main schedule ----
    # Heads are software-pipelined (exp(h) on ScalarE overlaps the PE running
    # scores(h+1)). ScalarE's per-head exp cost exceeds the PE's scores+PV
    # cost in every window, so each window's head ladder is padded with
    # filler PE work rationed to its ScalarE deficit: V for this window
    # (first — PV needs it), Q/K for the next, and the floating projection
    # halves weighted into the late windows where the deficit peaks.
    from collections import deque

    stgs = {}
    mask_tasks = deque()
    ot_tasks = deque()
    fill_q = deque()  # paced: next window's g0 Q/K units
    pri_q = deque()   # urgent: this window's V (pv needs it) and g1 Q/K
    unlocked_proj = deque()  # proj halves whose window's pv is fully emitted
    state = {"done": 0, "taps": 0, "units": 0, "wtaps": 1, "proj_budget": 0}

    def tap():
        state["taps"] += 1
        if pri_q:
            pri_q.popleft()()
            return
        want = min(state["units"],
                   (state["taps"] * state["units"]) // state["wtaps"] + 1)
        while state["done"] < want:
            if fill_q:
                fill_q.popleft()()
            elif unlocked_proj and state["proj_budget"] > 0:
                st, nh = unlocked_proj.popleft()
                proj_half(st, nh)
                state["proj_budget"] -= 1
            else:
                break
            state["done"] += 1

    # Minimal upfront PE work before the first scores: only the head-pair-0
    # Q/K rows of window 0, so ScalarE starts exp'ing as early as possible.
    warmup()
    qkv_qk_unit(0, 0, 0)
    qkv_qk_unit(0, 1, 0)

    PROJ_RATION = {0: 0, 1: 2, 2: 8, 3: 22}
    PV_LAG = 10  # pv trails scores by this many q-tile pairs (5 heads)
    pendq = deque()
    pv_done = {}

    unlock_stage = deque()

    def pop_pv():
        qw_, h_, pts_, tp_ = pendq.popleft()
        if h_ == 0 and tp_ == 0:
            while pri_q:  # pv(qw, 0) reads this window's V rows
                pri_q.popleft()()
        pv_half(qw_, h_, pts_, tp_)
        while unlock_stage:  # unlock lags one pv so the O^T copies land
            unlocked_proj.append(unlock_stage.popleft())
        done = pv_done[(qw_, tp_)] = pv_done.get((qw_, tp_), 0) + 1
        if done == 4:  # these two q-tiles now have all heads' O^T rows
            for st in range(4 * qw_ + 2 * tp_, 4 * qw_ + 2 * tp_ + 2):
                for nh in range(2):
                    unlock_stage.append((st, nh))
        tap()

    for qw in range(4):
        for wsel in range(2):                 # head-pair-1 Q/K, this window
            pri_q.append(lambda ws=wsel, w=qw: qkv_qk_unit(w, ws, 1))
        for st in range(4 * qw, 4 * qw + 4):  # V for this window
            pri_q.append(lambda s=st: qkv_v_unit(s))
        if qw < 3:
            for wsel in range(2):             # head-pair-0 Q/K, next window
                fill_q.append(lambda w=qw + 1, ws=wsel:
                              qkv_qk_unit(w, ws, 0))
        state["done"] = 0
        state["taps"] = 0
        state["proj_budget"] = PROJ_RATION[qw]
        state["units"] = len(fill_q) + min(PROJ_RATION[qw], 20)
        # drain the paced queue by ~80% of the window's taps
        state["wtaps"] = max((4 * (2 * qw + 2) + 4 - 6) * 4 // 5, 1)
        for h in range(4):
            if h == 2:  # scores(·, 2) reads head-pair-1 Q/K: force them in
                while pri_q:
                    pri_q.popleft()()
            pts = scores_head(qw, h, tap)
            pendq.append((qw, h, pts, 0))
            pendq.append((qw, h, pts, 1))
            while len(pendq) > PV_LAG:
                pop_pv()
            while mask_tasks:
                mask_tasks.popleft()()
        while pri_q:
            pri_q.popleft()()
        while fill_q:  # QKV must land before the next window needs it
            fill_q.popleft()()
    while pendq:
        pop_pv()
    while ot_tasks:
        ot_tasks.popleft()()
    while unlock_stage:
        unlocked_proj.append(unlock_stage.popleft())
    engs = ("dve", "act")
    k = 0
    while unlocked_proj:
        st, nh = unlocked_proj.popleft()
        proj_half(st, nh, copy_eng=engs[k % 2])
        k += 1


def build_bass():
    import concourse.tile as tile
    from concourse import bacc, mybir

    F32 = mybir.dt.float32
    BF16 = mybir.dt.bfloat16
    F8 = mybir.dt.float8e4
    nc = bacc.Bacc("TRN2", target_bir_lowering=False, debug=False,
                   enable_asserts=True, num_devices=N_CORES)
    x8 = nc.dram_tensor("x8", [128, 2, NCHUNK, S], F8, kind="ExternalInput").ap()
    w8q = nc.dram_tensor("w8q", [128, 2, NCHUNK, 256], F8, kind="ExternalInput").ap()
    w8k = nc.dram_tensor("w8k", [128, 2, NCHUNK, 256], F8, kind="ExternalInput").ap()
    w8v = nc.dram_tensor("w8v", [128, 2, NCHUNK, 256], F8, kind="ExternalInput").ap()
    wp = nc.dram_tensor("wp", [128, 2, D], BF16, kind="ExternalInput").ap()
    ident = nc.dram_tensor("ident", [128, 128], BF16, kind="ExternalInput").ap()
    dmask = nc.dram_tensor("dmask", [128, 4 * 512], BF16, kind="ExternalInput").ap()
    bq = nc.dram_tensor("bq", [128, 2], F32, kind="ExternalInput").ap()
    bk = nc.dram_tensor("bk", [128, 2], F32, kind="ExternalInput").ap()
    out_part = nc.dram_tensor("out_part", [S, D], BF16, kind="ExternalOutput").ap()

    with tile.TileContext(nc) as tc:
        with ExitStack() as ctx:
            _build_body(ctx, tc, x8, w8q, w8k, w8v, wp, ident, dmask, bq, bk,
                        out_part)
    nc.compile()
    return nc


# --------------------------------------------------------------------------
# host-side sharding
# --------------------------------------------------------------------------

def make_dmask():
    """dmask[k, j*512 + q] = 1.0 where q >= j*128 + k (diag blocks j=0..3)."""
    k = np.arange(128)[:, None]
    q = np.arange(512)[None, :]
    tiles = [(q >= j * 128 + k).astype(np.float32) for j in range(4)]
    return np.ascontiguousarray(np.concatenate(tiles, axis=1))


def _split_fp8(a):
    import ml_dtypes
    f8 = ml_dtypes.float8_e4m3
    hi = a.astype(f8)
    lo = (a - hi.astype(np.float32)).astype(f8)
    return hi, lo


def _planes(a, nchunk, bf=False):
    """[d, m] fp32 -> [128, 2, nchunk, m] fp8 hi/lo planes (d = c*128 + p)."""
    d, m = a.shape
    hi, lo = _split_fp8(a)
    arr = np.stack([hi.reshape(nchunk, 128, m), lo.reshape(nchunk, 128, m)], 0)
    return np.ascontiguousarray(arr.transpose(2, 0, 1, 3))


def host_inputs_for_core(core, x, qkv_w, proj_w, qkv_b):
    import ml_dtypes
    bf16 = ml_dtypes.bfloat16
    b, hg = core // 4, core % 4
    cols = slice(hg * 256, (hg + 1) * 256)
    bqs = qkv_b[0 * D:1 * D][cols].astype(np.float32)
    bks = qkv_b[1 * D:2 * D][cols].astype(np.float32)
    xt = np.ascontiguousarray(x[b].astype(np.float32).T)       # [D, S]
    return {
        "x8": _planes(xt, NCHUNK),
        "w8q": _planes(np.ascontiguousarray(qkv_w[:, 0 * D:1 * D][:, cols]) * AW, NCHUNK),
        "w8k": _planes(np.ascontiguousarray(qkv_w[:, 1 * D:2 * D][:, cols]) * AW, NCHUNK),
        "w8v": _planes(np.ascontiguousarray(qkv_w[:, 2 * D:3 * D][:, cols]) * AW, NCHUNK),
        "wp": np.ascontiguousarray(
            proj_w[hg * 256:(hg + 1) * 256, :].reshape(2, 128, D).transpose(1, 0, 2)
        ).astype(bf16),
        "ident": np.eye(128, dtype=np.float32).astype(bf16),
        "dmask": make_dmask().astype(bf16),
        "bq": np.ascontiguousarray(bqs.reshape(2, 128).T),
        "bk": np.ascontiguousarray(bks.reshape(2, 128).T),
    }


def _np_reference(x, mask, qkv_w, qkv_b, proj_w, proj_b):
    """numpy fallback, only used if inputs deviate from the expected
    causal-mask / shape contract."""
    b, s, d = x.shape
    hd = d // H_TOT
    qkv = x.astype(np.float32) @ qkv_w + qkv_b
    qkv = qkv.reshape(b, s, 3, H_TOT, hd).transpose(2, 0, 3, 1, 4)
    q, k, v = qkv[0], qkv[1], qkv[2]
    sc = np.einsum("bhqd,bhkd->bhqk", q, k) / np.sqrt(hd)
    sc = np.where(mask, sc, -np.inf)
    sc = sc - sc.max(axis=-1, keepdims=True)
    p = np.exp(sc)
    p = p / p.sum(axis=-1, keepdims=True)
    out = np.einsum("bhqk,bhkd->bhqd", p, v)
    out = out.transpose(0, 2, 1, 3).reshape(b, s, d)
    return (out @ proj_w + proj_b).astype(np.float32)


_NC_CACHE = []


def kernel(x, mask, qkv_w, qkv_b, proj_w, proj_b):
    x = np.asarray(x)
    mask = np.asarray(mask)
    qkv_w = np.asarray(qkv_w, dtype=np.float32)
    qkv_b = np.asarray(qkv_b, dtype=np.float32)
    proj_w = np.asarray(proj_w, dtype=np.float32)
    proj_b = np.asarray(proj_b, dtype=np.float32)

    causal = np.tril(np.ones((S, S), dtype=bool))
    ok_shapes = (x.shape == (B, S, D) and qkv_w.shape == (D, 3 * D)
                 and proj_w.shape == (D, D)
                 and mask.reshape(-1).shape == (S * S,))
    if not (ok_shapes and np.array_equal(mask.reshape(S, S), causal)):
        return _np_reference(x, mask, qkv_w, qkv_b, proj_w, proj_b)

    from concourse import bass_utils

    if not _NC_CACHE:
        _NC_CACHE.append(build_bass())
    nc = _NC_CACHE[0]

    in_maps = [host_inputs_for_core(c, x, qkv_w, proj_w, qkv_b)
               for c in range(N_CORES)]
    res = bass_utils.run_bass_kernel_spmd(nc, in_maps,
                                          core_ids=list(range(N_CORES)))
    parts = np.stack([res.results[c]["out_part"].astype(np.float32)
                      for c in range(N_CORES)])
    # v-bias correction: softmax weights sum to 1, so per head-group the V
    # bias adds exactly bv_hg @ proj_w_hg to every output row.
    bv_all = qkv_b[2 * D:3 * D]
    out = np.empty((B, S, D), np.float32)
    for b in range(B):
        out[b] = parts[b * 4:(b + 1) * 4].sum(axis=0) + proj_b \
            + bv_all @ proj_w
    return out
